# revision 1
# baseline (speedup 1.0000x reference)
"""Trainium2 Bass kernel for nn_Attention_org_10514079941402.

Math reduction: for each sample n (emb[n] is [T=8, D=2048]):
  G[n]      = emb[n] @ emb[n].T                      (8x8 Gram, contracts D)
  scores[h] = Wq[h] @ G[n] @ Wk[h].T / sqrt(T) + bias[h]
  probs     = softmax(instancenorm(scores))
  M[n]      = (1/H) * Wo @ (sum_h probs[h] @ Wv[h])  (8x8)
  out[n]    = M[n] @ emb[n]

Device pass 1 computes G for all samples (the only quadratic-in-emb part),
the tiny 8x8 chain runs on host, device pass 2 applies out = M @ emb via
block-diagonal stationary matrices (16 samples = 128 rows per matmul).
Data parallel over the leading N axis across 8 cores.

Walrus constraint: a PE instruction carries at most ONE sync wait. fp32
matmuls self-load weights (no separate LDWEIGHTS to spread waits over), so
bf16 LDWEIGHTS instructions are inserted as pure wait-carriers: they absorb
the cross-engine data waits, leaving each matmul only its PSUM-bank wait.
The garbage weights they load are irrelevant (fp32 matmuls self-load).
"""

import numpy as np

import concourse.bass as bass
import concourse.mybir as mybir
import concourse.tile as tile
from concourse.bass_utils import run_bass_kernel_spmd

PROFILE = False          # set by test harness; adds NTFF tracing
LAST_EXEC_NS = []        # per-launch HW exec times when PROFILE

N, T, D, H = 2048, 8, 2048, 4
NCORES = 8
NPC = N // NCORES            # 256 samples per core
GRP = 16                     # samples per 128-row group
GROUPS = NPC // GRP          # 16 groups per core
ROWS = NPC * T               # 2048 rows per core
EPS = 1e-5
FP = mybir.dt.float32
FPR = mybir.dt.float32r
BF = mybir.dt.bfloat16
NCHUNK = D // 128            # 16 transpose/gram chunks
NJ = D // 512                # 4 apply matmuls per group


def _carrier(nc, ap64):
    """bf16 LDWEIGHTS reading ap64 (a [128, 64] fp32 slice): absorbs the
    producer's semaphore wait onto a write-free PE instruction."""
    nc.tensor.ldweights(ap64.bitcast(BF))


def _strip_self_waits(nc):
    """Walrus accepts only ONE sync wait per engine instruction.

    1. Tile emits same-engine self-waits for slot releases; on strict-FIFO
       engines (DVE, ACT) program order already guarantees them - drop them.
    2. Any instruction still carrying >=2 waits gets the extras hoisted onto
       single-wait Drain instructions inserted just before it (same engine).
    """
    pref = {"EngineType.DVE": "DVE", "EngineType.ACT": "ACT",
            "EngineType.Activation": "ACT"}
    for blk in nc.m.functions[0].blocks:
        idx = 0
        insts = blk.instructions
        while idx < len(insts):
            inst = insts[idx]
            si = inst.sync_info
            if si is None:
                idx += 1
                continue
            waits = list(si.on_wait)
            if len(waits) < 2:
                idx += 1
                continue
            p = pref.get(str(inst.engine))
            if p is not None:
                keep = [w for w in waits if not w.ant_name.startswith(p)]
                if 1 <= len(keep) < len(waits):
                    waits = keep
            if len(waits) >= 2:
                for k, w in enumerate(waits[:-1]):
                    d = mybir.InstDrain(
                        name=f"{inst.name}_w{k}", ins=[], outs=[],
                        sync_info=mybir.SyncInfo(on_wait=[w], on_update=[]),
                    )
                    d.engine = inst.engine
                    insts.insert(idx, d)
                    idx += 1
                waits = [waits[-1]]
            inst.sync_info = mybir.SyncInfo(
                on_wait=waits, on_update=list(si.on_update)
            )
            idx += 1
    return nc


def _build_gram():
    nc = bass.Bass()
    emb = nc.dram_tensor("emb", [ROWS, D], FP, kind="ExternalInput")
    ident = nc.dram_tensor("ident", [128, 128], FP, kind="ExternalInput")
    gout = nc.dram_tensor("gout", [GROUPS, 128, 128], FP, kind="ExternalOutput")
    embr = emb[:, :].rearrange("(g p) d -> p g d", p=128)   # [128, GROUPS, D]
    with tile.TileContext(nc) as tc:
        with tc.tile_pool(name="const", bufs=1) as cpool, \
             tc.tile_pool(name="eb", bufs=1) as epool, \
             tc.tile_pool(name="et", bufs=2) as etpool, \
             tc.tile_pool(name="gsa", bufs=1) as gspool, \
             tc.tile_pool(name="etq0", bufs=1, space="PSUM") as psq0, \
             tc.tile_pool(name="etq1", bufs=1, space="PSUM") as psq1, \
             tc.tile_pool(name="etq2", bufs=1, space="PSUM") as psq2, \
             tc.tile_pool(name="etq3", bufs=1, space="PSUM") as psq3, \
             tc.tile_pool(name="gp", bufs=4, space="PSUM") as gppool:
            ident_sb = cpool.tile([128, 128], FP, name="ident_sb")
            nc.sync.dma_start(out=ident_sb[:], in_=ident[:, :])
            _carrier(nc, ident_sb[:, 0:64])
            # 4 chunked loads, 4 groups each, into 4 distinct tiles (no slot
            # reuse -> zero waits on load DMAs; <=8 HWDGE DMAs total keeps
            # every DMA on a private semaphore lane).
            GC = GROUPS // 4
            echunks = []
            for q in range(4):
                ec = epool.tile([128, GC, D], FP, name=f"ec{q}", tag=f"ec{q}")
                nc.sync.dma_start(out=ec[:], in_=embr[:, q * GC:(q + 1) * GC, :])
                echunks.append(ec)
            gs_all = gspool.tile([128, GROUPS, 128], FP, name="gs_all")

            def emit_transposes(g):
                e = echunks[g // GC][:, g % GC, :]
                _carrier(nc, e[:, 0:64])
                ets = etpool.tile([128, D], FP, name="ets", tag="ets")
                QC = NCHUNK // 4
                QW = D // 4
                for qi, pool in enumerate((psq0, psq1, psq2, psq3)):
                    etp = pool.tile([128, QW], FP, name=f"etq{qi}",
                                    tag=f"etq{qi}")
                    for ci in range(QC):
                        c = qi * QC + ci
                        nc.tensor.transpose(
                            out=etp[:, ci * 128:(ci + 1) * 128],
                            in_=e[:, c * 128:(c + 1) * 128],
                            identity=ident_sb[:],
                        )
                    dst = ets[:, qi * QW:(qi + 1) * QW]
                    if qi % 2 == 0:
                        nc.vector.tensor_copy(dst, etp[:])
                    else:
                        nc.scalar.copy(dst, etp[:])
                return ets

            def emit_grams(g, ets):
                _carrier(nc, ets[:, 2 * (D // 4):2 * (D // 4) + 64])
                _carrier(nc, ets[:, 3 * (D // 4):3 * (D // 4) + 64])
                gp = gppool.tile([128, 128], FP, name="gp", tag="gp")
                for c in range(NCHUNK):
                    nc.tensor.matmul(
                        gp[:],
                        ets[:, c * 128:(c + 1) * 128],
                        ets[:, c * 128:(c + 1) * 128],
                        start=(c == 0),
                        stop=(c == NCHUNK - 1),
                    )
                nc.vector.tensor_copy(gs_all[:, g, :], gp[:])

            # One-group software pipeline: group g+1's transposes are emitted
            # before group g's gram matmuls, so PE never stalls waiting for
            # the PSUM->SBUF copies of the group it is about to contract.
            pend = (0, emit_transposes(0))
            for g in range(1, GROUPS):
                ets = emit_transposes(g)
                emit_grams(*pend)
                pend = (g, ets)
            emit_grams(*pend)
            nc.sync.dma_start(
                out=gout[:, :, :].rearrange("g p c -> p g c"), in_=gs_all[:]
            )
    return _strip_self_waits(nc)


def _build_apply():
    nc = bass.Bass()
    emb = nc.dram_tensor("emb", [ROWS, D], FP, kind="ExternalInput")
    bd = nc.dram_tensor("bd", [GROUPS, 128, 128], FP, kind="ExternalInput")
    outp = nc.dram_tensor("outp", [ROWS, D], FP, kind="ExternalOutput")
    embr = emb[:, :].rearrange("(g p) d -> p g d", p=128)
    outr = outp[:, :].rearrange("(g p) d -> p g d", p=128)
    with tile.TileContext(nc) as tc:
        with tc.tile_pool(name="bda", bufs=1) as bdapool, \
             tc.tile_pool(name="eb", bufs=1) as epool, \
             tc.tile_pool(name="os2", bufs=4) as ospool, \
             tc.tile_pool(name="op", bufs=2, space="PSUM") as oppool:
            bd_sb = bdapool.tile([128, GROUPS, 128], FP, name="bd_sb")
            nc.sync.dma_start(
                out=bd_sb[:], in_=bd[:, :, :].rearrange("g p c -> p g c")
            )
            _carrier(nc, bd_sb[:, 0, 0:64])
            GC = GROUPS // 4
            echunks = []
            for q in range(4):
                ec = epool.tile([128, GC, D], FP, name=f"ec{q}", tag=f"ec{q}")
                nc.sync.dma_start(out=ec[:], in_=embr[:, q * GC:(q + 1) * GC, :])
                echunks.append(ec)
            os_hist = []
            os2 = None
            for g in range(GROUPS):
                e = echunks[g // GC][:, g % GC, :]
                _carrier(nc, e[:, 0:64])
                os2 = ospool.tile([128, D], FP, name="os2", tag="os2")
                os_hist.append(os2)
                if g >= 2:
                    prev = os_hist[g - 2]
                    # op slot reuse: absorb both copy-engine dependencies.
                    _carrier(nc, prev[:, 0:64])
                    _carrier(nc, prev[:, D // 2:D // 2 + 64])
                for hhalf in range(2):
                    oph = oppool.tile([128, D // 2], FP, name="oph",
                                      tag=f"oph{hhalf}")
                    for jj in range(2):
                        j = hhalf * 2 + jj
                        nc.tensor.matmul(
                            oph[:, jj * 512:(jj + 1) * 512],
                            bd_sb[:, g, :],
                            e[:, j * 512:(j + 1) * 512],
                            start=True,
                            stop=True,
                        )
                    dst = os2[:, hhalf * (D // 2):(hhalf + 1) * (D // 2)]
                    if hhalf == 0:
                        nc.vector.tensor_copy(dst, oph[:])
                    else:
                        nc.scalar.copy(dst, oph[:])
                nc.sync.dma_start(out=outr[:, g, :], in_=os2[:])
    return _strip_self_waits(nc)


def _host_small_math(Gn, Wq, Wk, Wv, Wo, rel_table):
    """Gn [N,T,T] -> M [N,T,T] with out[n] = M[n] @ emb[n]."""
    scale = np.float32(1.0 / np.sqrt(T))
    scores = np.einsum("hta,nab,hsb->nhts", Wq, Gn, Wk) * scale
    idx = np.arange(T)[:, None] - np.arange(T)[None, :] + T - 1
    bias = rel_table[idx]                      # [T,T,H]
    scores = scores + bias.transpose(2, 0, 1)[None]
    mu = scores.mean(axis=(2, 3), keepdims=True)
    var = scores.var(axis=(2, 3), keepdims=True)
    scores = (scores - mu) / np.sqrt(var + EPS)
    scores = scores - scores.max(axis=-1, keepdims=True)
    ex = np.exp(scores)
    probs = ex / ex.sum(axis=-1, keepdims=True)
    A = np.einsum("nhts,hsu->ntu", probs, Wv) / np.float32(H)
    M = np.einsum("tu,nus->nts", Wo, A)
    return M.astype(np.float32)


def kernel(emb, Wq, Wk, Wv, Wo, rel_table):
    emb = np.ascontiguousarray(emb, dtype=np.float32)
    Wq = np.asarray(Wq, np.float32)
    Wk = np.asarray(Wk, np.float32)
    Wv = np.asarray(Wv, np.float32)
    Wo = np.asarray(Wo, np.float32)
    rel_table = np.asarray(rel_table, np.float32)

    embc = emb.reshape(NCORES, ROWS, D)
    ident = np.eye(128, dtype=np.float32)
    core_ids = list(range(NCORES))

    del LAST_EXEC_NS[:]
    nc1 = _build_gram()
    r1 = run_bass_kernel_spmd(
        nc1, [{"emb": embc[i], "ident": ident} for i in range(NCORES)], core_ids,
        trace=PROFILE,
    )
    if PROFILE:
        LAST_EXEC_NS.append(r1.exec_time_ns)
    G = np.stack([r1.results[i]["gout"] for i in range(NCORES)])
    # [C, GROUPS, 128, 128] -> diagonal 8x8 blocks -> [N, T, T]
    Gb = G.reshape(NCORES, GROUPS, GRP, T, GRP, T)
    Gn = np.einsum("cgbtbs->cgbts", Gb).reshape(N, T, T)

    M = _host_small_math(Gn, Wq, Wk, Wv, Wo, rel_table)

    Mn = M.reshape(NCORES, GROUPS, GRP, T, T)
    bd = np.zeros((NCORES, GROUPS, GRP, T, GRP, T), np.float32)
    for b in range(GRP):
        # BD[(b,s),(b,t)] = M[b][t,s]
        bd[:, :, b, :, b, :] = Mn[:, :, b].swapaxes(-1, -2)
    bd = bd.reshape(NCORES, GROUPS, 128, 128)

    nc2 = _build_apply()
    r2 = run_bass_kernel_spmd(
        nc2,
        [{"emb": embc[i], "bd": bd[i]} for i in range(NCORES)],
        core_ids,
        trace=PROFILE,
    )
    if PROFILE:
        LAST_EXEC_NS.append(r2.exec_time_ns)
    out = np.stack([r2.results[i]["outp"] for i in range(NCORES)])
    return out.reshape(N, T, D)



# revision 13
# speedup vs baseline: 1.3962x; 1.3962x over previous
"""Trainium2 Bass kernel for nn_Attention_org_10514079941402 (fused 1-launch).

Math per sample n (emb[n] is [T=8, D=2048]):
  G[n]      = emb[n] @ emb[n].T                       (8x8 Gram, contracts D)
  scores[h] = Wq[h] @ G[n] @ Wk[h].T / sqrt(T) + bias[h]
  probs     = softmax(instancenorm(scores))           (mean-shift drops out of
                                                       softmax; only the 1/std
                                                       scale + mu*r bias matter)
  M[n]      = (1/H) * Wo @ (sum_h probs[h] @ Wv[h])   (8x8)
  out[n]    = M[n] @ emb[n]

Everything runs in ONE device launch per core (data parallel over N):
  1. emb resident in SBUF; per 128-row group: PE transposes (fp32r) ->
     bf16 Et -> PE gram -> gs_all.
  2. Per 128-sample tile: diagonal 8x8 blocks of the group-grams are pulled
     into a per-sample layout gvec[sample, 64] with 16 pure-stride SBUF->SBUF
     DMAs (one per sample-in-group index; BIR forbids APs whose dims couple
     partition+byte strides, but fixing i per DMA keeps every stride pure).
     scores = W2aug^T @ [gvec|1] on PE (the bias rides as a 65th contraction
     row); instancenorm stats + exp fused into 4 ACT activations
     (exp(x*r - mu*r)); softmax row-sums on DVE; probs^T via PE; M-vectors
     via 2 accumulating PE matmuls against W3; scattered into a zeroed
     block-diagonal BD_all with 16 more pure-stride DMAs.
  3. Per group: out = BD^T @ emb on PE in fp32r, written out as fp16
     (halves the store traffic; host casts back to fp32).

Walrus constraint: a PE instruction carries at most ONE sync wait. bf16
LDWEIGHTS instructions are inserted as pure wait-carriers; any instruction
still carrying >=2 waits gets the extras hoisted onto Drain instructions.
"""

import numpy as np

import concourse.bass as bass
import concourse.mybir as mybir
import concourse.tile as tile
from concourse.bass_utils import run_bass_kernel_spmd

PROFILE = False          # set by test harness
LAST_EXEC_NS = []        # per-launch HW exec times when PROFILE

N, T, D, H = 2048, 8, 2048, 4
NCORES = 8
NPC = N // NCORES            # 256 samples per core
GRP = 16                     # samples per 128-row group
GROUPS = NPC // GRP          # 16 groups per core
GPT = 8                      # groups per 128-sample tile
TILES = GROUPS // GPT        # 2 tiles per core
ROWS = NPC * T               # 2048 rows per core
EPS = 1e-5
FP = mybir.dt.float32
FPR = mybir.dt.float32r
BF = mybir.dt.bfloat16
F16 = mybir.dt.float16
NCHUNK = D // 128            # 16 transpose/gram chunks per group


def _carrier(nc, ap64):
    """bf16 LDWEIGHTS reading ap64 (a [128, 64] fp32 slice): absorbs the
    producer's semaphore wait onto a write-free PE instruction."""
    nc.tensor.ldweights(ap64.bitcast(BF))


def _strip_self_waits(nc):
    """Walrus accepts only ONE sync wait per engine instruction.

    1. Tile emits same-engine self-waits for slot releases; on strict-FIFO
       engines (DVE, ACT) program order already guarantees them - drop them.
    2. Any instruction still carrying >=2 waits gets the extras hoisted onto
       single-wait Drain instructions inserted just before it (same engine).
    """
    pref = {"EngineType.DVE": "DVE", "EngineType.ACT": "ACT",
            "EngineType.Activation": "ACT"}
    for blk in nc.m.functions[0].blocks:
        idx = 0
        insts = blk.instructions
        while idx < len(insts):
            inst = insts[idx]
            si = inst.sync_info
            if si is None:
                idx += 1
                continue
            waits = list(si.on_wait)
            if len(waits) < 2:
                idx += 1
                continue
            p = pref.get(str(inst.engine))
            if p is not None:
                keep = [w for w in waits if not w.ant_name.startswith(p)]
                if 1 <= len(keep) < len(waits):
                    waits = keep
            if len(waits) >= 2:
                for k, w in enumerate(waits[:-1]):
                    d = mybir.InstDrain(
                        name=f"{inst.name}_w{k}", ins=[], outs=[],
                        sync_info=mybir.SyncInfo(on_wait=[w], on_update=[]),
                    )
                    d.engine = inst.engine
                    insts.insert(idx, d)
                    idx += 1
                waits = [waits[-1]]
            inst.sync_info = mybir.SyncInfo(
                on_wait=waits, on_update=list(si.on_update)
            )
            idx += 1
    return nc


def _build_fused():
    nc = bass.Bass()
    emb = nc.dram_tensor("emb", [ROWS, D], FPR, kind="ExternalInput")
    ident = nc.dram_tensor("ident", [128, 128], FPR, kind="ExternalInput")
    w2aug = nc.dram_tensor("w2aug", [65, 256], FPR, kind="ExternalInput")
    w3c = nc.dram_tensor("w3c", [128, 128], FPR, kind="ExternalInput")
    outp = nc.dram_tensor("outp", [ROWS, D], F16, kind="ExternalOutput")
    gw = nc.dram_tensor("gw", [128, GROUPS, 128], FPR, kind="Internal")
    mvd = nc.dram_tensor("mvd", [NPC, 64], FPR, kind="Internal")
    bdz = nc.dram_tensor("bdz", [GROUPS, 128, 128], FPR, kind="ExternalInput")
    onesr = nc.dram_tensor("onesr", [1, 128], FPR, kind="ExternalInput")
    embr = emb[:, :].rearrange("(g p) d -> p g d", p=128)   # [128, GROUPS, D]
    outr = outp[:, :].rearrange("(g p) d -> p g d", p=128)
    AX = mybir.AxisListType
    OP = mybir.AluOpType
    AF = mybir.ActivationFunctionType

    with tile.TileContext(nc) as tc:
        with tc.tile_pool(name="const", bufs=1) as cpool, \
             tc.tile_pool(name="embp", bufs=1) as epool, \
             tc.tile_pool(name="etp", bufs=2) as etpool, \
             tc.tile_pool(name="gsp", bufs=1) as gspool, \
             tc.tile_pool(name="bdp", bufs=1) as bdpool, \
             tc.tile_pool(name="sml", bufs=2) as smpool, \
             tc.tile_pool(name="osb", bufs=2) as opool, \
             tc.tile_pool(name="tq", bufs=3, space="PSUM") as tqpool, \
             tc.tile_pool(name="gp", bufs=1, space="PSUM") as gppool, \
             tc.tile_pool(name="psm", bufs=2, space="PSUM") as psmall, \
             tc.tile_pool(name="app", bufs=2, space="PSUM") as apool:
            ident_sb = cpool.tile([128, 128], FPR, name="ident_sb")
            nc.sync.dma_start(out=ident_sb[:], in_=ident[:, :])
            w2_sb = cpool.tile([128, 256], FPR, name="w2_sb")
            nc.sync.dma_start(out=w2_sb[0:65, :], in_=w2aug[:, :])
            w3_sb = cpool.tile([128, 128], FPR, name="w3_sb")
            nc.sync.dma_start(out=w3_sb[:], in_=w3c[:, :])
            eps_sb = cpool.tile([128, 1], FP, name="eps_sb")
            nc.vector.memset(eps_sb[:], EPS)
            ones_sb = cpool.tile([1, 128], FPR, name="ones_sb")
            nc.sync.dma_start(out=ones_sb[:], in_=onesr[:, :])
            _carrier(nc, ident_sb[:, 0:64])

            emb_sb = epool.tile([128, GROUPS, D], FPR, name="emb_sb")
            for g in range(GROUPS):
                nc.sync.dma_start(out=emb_sb[:, g, :], in_=embr[:, g, :])

            bd_all = bdpool.tile([128, GROUPS, 128], FPR, name="bd_all")
            gs_all = gspool.tile([128, GROUPS, 128], FPR, name="gs_all")

            def emit_transposes(g):
                e = emb_sb[:, g, :]
                _carrier(nc, e[:, 0:64])
                ets = etpool.tile([128, D], BF, name="ets", tag="ets")
                QW = D // 4
                for qi in range(4):
                    etq = tqpool.tile([128, QW], FPR, name="etq", tag="tq")
                    for ci in range(QW // 128):
                        c = qi * (QW // 128) + ci
                        nc.tensor.transpose(
                            out=etq[:, ci * 128:(ci + 1) * 128],
                            in_=e[:, c * 128:(c + 1) * 128],
                            identity=ident_sb[:],
                        )
                    dst = ets[:, qi * QW:(qi + 1) * QW]
                    if qi % 2 == 0:
                        nc.vector.tensor_copy(dst, etq[:])
                    else:
                        nc.scalar.copy(dst, etq[:])
                return ets

            def emit_grams(g, ets):
                _carrier(nc, ets[:, 0:128].bitcast(FP))
                _carrier(nc, ets[:, D // 2:D // 2 + 128].bitcast(FP))
                gp = gppool.tile([128, 128], FP, name="gp", tag="gp")
                for c in range(NCHUNK):
                    nc.tensor.matmul(
                        gp[:],
                        ets[:, c * 128:(c + 1) * 128],
                        ets[:, c * 128:(c + 1) * 128],
                        start=(c == 0),
                        stop=(c == NCHUNK - 1),
                    )
                nc.vector.tensor_copy(gs_all[:, g, :], gp[:])

            def emit_gram_phase(t0):
                pend = (t0, emit_transposes(t0))
                for g in range(t0 + 1, t0 + GPT):
                    ets = emit_transposes(g)
                    emit_grams(*pend)
                    pend = (g, ets)
                emit_grams(*pend)

            def emit_small(t):
                t0 = t * GPT
                # --- diag gather: gvec[16*gg+i, 8a+b] = G[n][a, b] ---
                # (BIR allows partition steps only in an SBUF AP's outermost
                # dim, so the diagonal is taken on the DRAM side of a bounce.)
                nc.sync.dma_start(out=gw[:, t0:t0 + GPT, :],
                                  in_=gs_all[:, t0:t0 + GPT, :])
                gvec = smpool.tile([128, 64], FPR, name="gvec", tag="gvec")
                for i in range(GRP):
                    src = bass.AP(
                        gw, i * (8 * GROUPS * 128) + t0 * 128 + 8 * i,
                        [[128, GPT], [GROUPS * 128, 8], [1, 8]],
                    )
                    dst = bass.AP(
                        gvec.tensor, gvec.offset + i * 64,
                        [[16 * 64, GPT], [8, 8], [1, 8]],
                    )
                    nc.sync.dma_start(out=dst, in_=src)
                # --- gvt = [gvec^T; ones] (65 x 128) ---
                gvtT = psmall.tile([64, 128], FPR, name="gvtT", tag="psm")
                _carrier(nc, gvec[:, 0:64])
                nc.tensor.transpose(
                    out=gvtT[:],
                    in_=gvec[:],
                    identity=ident_sb[:],
                )
                gvt = smpool.tile([128, 128], FPR, name="gvt", tag="gvt")
                nc.vector.tensor_copy(gvt[0:64, :], gvtT[:])
                nc.vector.tensor_copy(gvt[64:65, :], ones_sb[0:1, :])
                # --- scores = [gvec|1] @ W2aug  ([128 samples, 256 hts]) ---
                scores = psmall.tile([128, 256], FP, name="scores", tag="psm")
                _carrier(nc, gvt[:, 0:64])
                nc.tensor.matmul(
                    scores[:],
                    gvt[0:65, :],
                    w2_sb[0:65, :],
                    start=True, stop=True,
                )
                # --- instancenorm stats per (sample, head) over 64 elems ---
                mu = smpool.tile([128, 4], FP, name="mu", tag="mu")
                nc.vector.tensor_reduce(
                    out=mu[:],
                    in_=scores[:].rearrange("p (h x) -> p h x", h=4),
                    axis=AX.X, op=OP.add,
                )
                sq = smpool.tile([128, 4], FP, name="sq", tag="sq")
                pn = smpool.tile([128, 256], FPR, name="pn", tag="pn")
                probs = smpool.tile([128, 256], FP, name="probs", tag="probs")
                for h in range(4):
                    nc.scalar.activation(
                        out=probs[:, h * 64:(h + 1) * 64],
                        in_=scores[:, h * 64:(h + 1) * 64],
                        func=AF.Square,
                        accum_out=sq[:, h:h + 1],
                    )
                m2 = smpool.tile([128, 4], FP, name="m2", tag="m2")
                # m2 = (mu/4096) * mu = mu^2/64^2 ; var = sq/64 - m2
                nc.vector.scalar_tensor_tensor(
                    out=m2[:], in0=mu[:], scalar=1.0 / 4096.0, in1=mu[:],
                    op0=OP.mult, op1=OP.mult,
                )
                var = smpool.tile([128, 4], FP, name="var", tag="var")
                nc.vector.scalar_tensor_tensor(
                    out=var[:], in0=sq[:], scalar=1.0 / 64.0, in1=m2[:],
                    op0=OP.mult, op1=OP.subtract,
                )
                std = smpool.tile([128, 4], FP, name="std", tag="std")
                nc.scalar.activation(out=std[:], in_=var[:], func=AF.Sqrt,
                                     bias=eps_sb[:, 0:1])
                r = smpool.tile([128, 4], FP, name="r", tag="r")
                nc.vector.reciprocal(r[:], std[:])
                rb = smpool.tile([128, 4], FP, name="rb", tag="rb")
                # rb = (mu * -1/64) * r = -mu_mean * r
                nc.vector.scalar_tensor_tensor(
                    out=rb[:], in0=mu[:], scalar=-1.0 / 64.0, in1=r[:],
                    op0=OP.mult, op1=OP.mult,
                )
                # --- probs = exp(x*r - mu*r) fused on ACT ---
                for h in range(4):
                    nc.scalar.activation(
                        out=probs[:, h * 64:(h + 1) * 64],
                        in_=scores[:, h * 64:(h + 1) * 64],
                        func=AF.Exp,
                        bias=rb[:, h:h + 1], scale=r[:, h:h + 1],
                    )
                # --- softmax row sums (segments of 8) + divide ---
                z = smpool.tile([128, 32], FP, name="z", tag="z")
                nc.vector.tensor_reduce(
                    out=z[:],
                    in_=probs[:].rearrange("p (w s) -> p w s", w=32),
                    axis=AX.X, op=OP.add,
                )
                rz = smpool.tile([128, 32], FP, name="rz", tag="rz")
                nc.vector.reciprocal(rz[:], z[:])
                nc.vector.tensor_tensor(
                    out=pn[:].rearrange("p (w s) -> p w s", w=32),
                    in0=probs[:].rearrange("p (w s) -> p w s", w=32),
                    in1=rz[:].unsqueeze(-1).broadcast_to((128, 32, 8)),
                    op=OP.mult,
                )
                # --- mv[n, 8s+t] = M[n,t,s] via probs^T then W3 ---
                pt = smpool.tile([128, 256], FPR, name="pt", tag="pt")
                for half in range(2):
                    ptT = psmall.tile([128, 128], FPR, name="ptT", tag="psm")
                    _carrier(nc, pn[:, half * 128:half * 128 + 64])
                    nc.tensor.transpose(
                        out=ptT[:],
                        in_=pn[:, half * 128:(half + 1) * 128],
                        identity=ident_sb[:],
                    )
                    if half == 0:
                        nc.vector.tensor_copy(pt[:, 0:128], ptT[:])
                    else:
                        nc.scalar.copy(pt[:, 128:256], ptT[:])
                mvp = psmall.tile([128, 64], FP, name="mvp", tag="psm")
                _carrier(nc, pt[:, 0:64])
                _carrier(nc, pt[:, 128:192])
                for half in range(2):
                    nc.tensor.matmul(
                        mvp[:],
                        pt[:, half * 128:(half + 1) * 128],
                        w3_sb[:, half * 64:(half + 1) * 64],
                        start=(half == 0), stop=(half == 1),
                    )
                mv = smpool.tile([128, 64], FPR, name="mv", tag="mv")
                nc.vector.tensor_copy(mv[:], mvp[:])
                # --- diag scatter: BD[8b+s, g, 8b+t] = mv[16*gg+b, 8s+t] ---
                # mv -> DRAM, diagonal placed DRAM->DRAM into pre-zeroed bdz,
                # then the tile half of bdz is loaded as block-diag matrices.
                nc.sync.dma_start(out=mvd[t * 128:(t + 1) * 128, :], in_=mv[:])
                for b in range(GRP):
                    src = bass.AP(
                        mvd, (t * 128 + b) * 64,
                        [[8, 8], [16 * 64, GPT], [1, 8]],
                    )
                    dst = bass.AP(
                        bdz, t0 * 16384 + b * 8 * 128 + 8 * b,
                        [[128, 8], [16384, GPT], [1, 8]],
                    )
                    nc.sync.dma_start(out=dst, in_=src)
                nc.sync.dma_start(
                    out=bd_all[:, t0:t0 + GPT, :],
                    in_=bdz[t0:t0 + GPT, :, :].rearrange("g p c -> p g c"),
                )

            def emit_apply(t):
                t0 = t * GPT
                for g in range(t0, t0 + GPT):
                    os = opool.tile([128, D], F16, name="os", tag="os")
                    _carrier(nc, bd_all[:, g, 0:64])
                    for jj in range(4):
                        ap_ps = apool.tile([128, 512], FP, name="ap_ps",
                                           tag="app")
                        nc.tensor.matmul(
                            ap_ps[:],
                            bd_all[:, g, :],
                            emb_sb[:, g, jj * 512:(jj + 1) * 512],
                            start=True, stop=True,
                        )
                        dst = os[:, jj * 512:(jj + 1) * 512]
                        if jj % 2 == 0:
                            nc.vector.tensor_copy(dst, ap_ps[:])
                        else:
                            nc.scalar.copy(dst, ap_ps[:])
                    nc.sync.dma_start(out=outr[:, g, :], in_=os[:])

            emit_gram_phase(0)
            emit_small(0)
            emit_gram_phase(GPT)
            emit_apply(0)
            emit_small(1)
            emit_apply(1)
    return _strip_self_waits(nc)


def _host_consts(Wq, Wk, Wv, Wo, rel_table):
    scale = np.float32(1.0 / np.sqrt(T))
    w2 = scale * np.einsum("hta,hsb->abhts", Wq, Wk)
    w2aug = np.zeros((65, 256), np.float32)
    w2aug[:64] = w2.reshape(64, 256)
    idx = np.arange(T)[:, None] - np.arange(T)[None, :] + T - 1
    bias = rel_table[idx]                       # [t, s, H]
    w2aug[64] = bias.transpose(2, 0, 1).reshape(256)
    # w3[64h+8u+q, 8s+t] = Wo[t,u]*Wv[h,q,s]/H
    w3 = np.einsum("tu,hqs->huqst", Wo, Wv).reshape(256, 64) / np.float32(H)
    w3c = np.zeros((128, 128), np.float32)
    w3c[:, 0:64] = w3[0:128]
    w3c[:, 64:128] = w3[128:256]
    return w2aug.astype(np.float32), w3c.astype(np.float32)


def kernel(emb, Wq, Wk, Wv, Wo, rel_table):
    emb = np.ascontiguousarray(emb, dtype=np.float32)
    Wq = np.asarray(Wq, np.float32)
    Wk = np.asarray(Wk, np.float32)
    Wv = np.asarray(Wv, np.float32)
    Wo = np.asarray(Wo, np.float32)
    rel_table = np.asarray(rel_table, np.float32)

    embc = emb.reshape(NCORES, ROWS, D)
    ident = np.eye(128, dtype=np.float32)
    bdz = np.zeros((GROUPS, 128, 128), np.float32)
    onesr_np = np.ones((1, 128), np.float32)
    w2aug, w3c = _host_consts(Wq, Wk, Wv, Wo, rel_table)
    core_ids = list(range(NCORES))

    del LAST_EXEC_NS[:]
    ncf = _build_fused()
    r = run_bass_kernel_spmd(
        ncf,
        [{"emb": embc[i], "ident": ident, "w2aug": w2aug, "w3c": w3c,
          "bdz": bdz, "onesr": onesr_np}
         for i in range(NCORES)],
        core_ids,
        trace=PROFILE,
    )
    if PROFILE:
        LAST_EXEC_NS.append(r.exec_time_ns)
    out = np.stack([np.asarray(r.results[i]["outp"]) for i in range(NCORES)])
    return out.reshape(N, T, D).astype(np.float32)


# revision 41
# speedup vs baseline: 1.7326x; 1.2410x over previous
"""Trainium2 Bass kernel for nn_Attention_org_10514079941402 (fused 1-launch).

Math per sample n (emb[n] is [T=8, D=2048]):
  G[n]      = emb[n] @ emb[n].T                       (8x8 Gram, contracts D)
  scores[h] = Wq[h] @ G[n] @ Wk[h].T / sqrt(T) + bias[h]
  probs     = softmax(instancenorm(scores))           (mean-shift drops out of
                                                       softmax; only the 1/std
                                                       scale + mu*r bias matter)
  M[n]      = (1/H) * Wo @ (sum_h probs[h] @ Wv[h])   (8x8)
  out[n]    = M[n] @ emb[n]

Everything runs in ONE device launch per core (data parallel over N):
  1. emb resident in SBUF; per 128-row group: PE transposes (fp32r) ->
     bf16 Et -> PE gram -> gs_all.
  2. Per 128-sample tile: diagonal 8x8 blocks of the group-grams are pulled
     into a per-sample layout gvec[sample, 64] with 16 pure-stride SBUF->SBUF
     DMAs (one per sample-in-group index; BIR forbids APs whose dims couple
     partition+byte strides, but fixing i per DMA keeps every stride pure).
     scores = W2aug^T @ [gvec|1] on PE (the bias rides as a 65th contraction
     row); instancenorm stats + exp fused into 4 ACT activations
     (exp(x*r - mu*r)); softmax row-sums on DVE; probs^T via PE; M-vectors
     via 2 accumulating PE matmuls against W3; scattered into a zeroed
     block-diagonal BD_all with 16 more pure-stride DMAs.
  3. Per group: out = BD^T @ emb on PE in fp32r, written out as fp16
     (halves the store traffic; host casts back to fp32).

Walrus constraint: a PE instruction carries at most ONE sync wait. bf16
LDWEIGHTS instructions are inserted as pure wait-carriers; any instruction
still carrying >=2 waits gets the extras hoisted onto Drain instructions.
"""

import numpy as np

import concourse.bass as bass
import concourse.mybir as mybir
import concourse.tile as tile
from concourse.bass_utils import run_bass_kernel_spmd

PROFILE = False          # set by test harness
LAST_EXEC_NS = []        # per-launch HW exec times when PROFILE
QUEUE_SPLIT = True       # spread tiny-DMA issue across SP/Pool/ACT queues
POOL_OUTS = False        # issue output DMAs from the Pool (SWDGE) queue

N, T, D, H = 2048, 8, 2048, 4
NCORES = 8
NPC = N // NCORES            # 256 samples per core
GRP = 16                     # samples per 128-row group
GROUPS = NPC // GRP          # 16 groups per core
GPT = 8                      # groups per 128-sample tile
TILES = GROUPS // GPT        # 2 tiles per core
ROWS = NPC * T               # 2048 rows per core
EPS = 1e-5
FP = mybir.dt.float32
FPR = mybir.dt.float32r
BF = mybir.dt.bfloat16
F16 = mybir.dt.float16
NCHUNK = D // 128            # 16 transpose/gram chunks per group


def _carrier(nc, ap64):
    """bf16 LDWEIGHTS reading ap64 (a [128, 64] fp32 slice): absorbs the
    producer's semaphore wait onto a write-free PE instruction."""
    nc.tensor.ldweights(ap64.bitcast(BF))


def _strip_self_waits(nc):
    """Walrus accepts only ONE sync wait per engine instruction.

    1. Tile emits same-engine self-waits for slot releases; on strict-FIFO
       engines (DVE, ACT) program order already guarantees them - drop them.
    2. Any instruction still carrying >=2 waits gets the extras hoisted onto
       single-wait Drain instructions inserted just before it (same engine).
    """
    pref = {"EngineType.DVE": "DVE", "EngineType.ACT": "ACT",
            "EngineType.Activation": "ACT"}
    for blk in nc.m.functions[0].blocks:
        idx = 0
        insts = blk.instructions
        while idx < len(insts):
            inst = insts[idx]
            si = inst.sync_info
            if si is None:
                idx += 1
                continue
            waits = list(si.on_wait)
            if len(waits) < 2:
                idx += 1
                continue
            p = pref.get(str(inst.engine))
            if p is not None:
                keep = [w for w in waits if not w.ant_name.startswith(p)]
                if 1 <= len(keep) < len(waits):
                    waits = keep
            if len(waits) >= 2:
                for k, w in enumerate(waits[:-1]):
                    d = mybir.InstDrain(
                        name=f"{inst.name}_w{k}", ins=[], outs=[],
                        sync_info=mybir.SyncInfo(on_wait=[w], on_update=[]),
                    )
                    d.engine = inst.engine
                    insts.insert(idx, d)
                    idx += 1
                waits = [waits[-1]]
            inst.sync_info = mybir.SyncInfo(
                on_wait=waits, on_update=list(si.on_update)
            )
            idx += 1
    return nc


def _build_fused(marks=None):
    def mark(label):
        if marks is not None:
            marks.append((label, nc.next_id()))
    nc = bass.Bass()
    emb = nc.dram_tensor("emb", [ROWS, D], FPR, kind="ExternalInput")
    ident = nc.dram_tensor("ident", [128, 128], FPR, kind="ExternalInput")
    w2aug = nc.dram_tensor("w2aug", [65, 256], FPR, kind="ExternalInput")
    w3c = nc.dram_tensor("w3c", [128, 128], FPR, kind="ExternalInput")
    outp = nc.dram_tensor("outp", [ROWS, D], F16, kind="ExternalOutput")
    gvd = nc.dram_tensor("gvd", [NPC, 64], FPR, kind="Internal")
    mvd = nc.dram_tensor("mvd", [NPC, 64], FPR, kind="Internal")
    onesr = nc.dram_tensor("onesr", [1, 128], FPR, kind="ExternalInput")
    embr = emb[:, :].rearrange("(g p) d -> p g d", p=128)   # [128, GROUPS, D]
    outr = outp[:, :].rearrange("(g p) d -> p g d", p=128)
    AX = mybir.AxisListType
    OP = mybir.AluOpType
    AF = mybir.ActivationFunctionType

    with tile.TileContext(nc) as tc:
        with tc.tile_pool(name="const", bufs=1) as cpool, \
             tc.tile_pool(name="embp", bufs=1) as epool, \
             tc.tile_pool(name="etp", bufs=2) as etpool, \
             tc.tile_pool(name="gsp", bufs=1) as gspool, \
             tc.tile_pool(name="bdp", bufs=1) as bdpool, \
             tc.tile_pool(name="sml", bufs=2) as smpool, \
             tc.tile_pool(name="osb", bufs=6) as opool, \
             tc.tile_pool(name="tq", bufs=5, space="PSUM") as tqpool, \
             tc.tile_pool(name="gp", bufs=1, space="PSUM") as gppool, \
             tc.tile_pool(name="psm", bufs=2, space="PSUM") as psmall:
            ident_sb = cpool.tile([128, 128], FPR, name="ident_sb")
            nc.sync.dma_start(out=ident_sb[:], in_=ident[:, :])
            w2_sb = cpool.tile([128, 256], FPR, name="w2_sb")
            nc.sync.dma_start(out=w2_sb[0:65, :], in_=w2aug[:, :])
            w3_sb = cpool.tile([128, 128], FPR, name="w3_sb")
            nc.sync.dma_start(out=w3_sb[:], in_=w3c[:, :])
            eps_sb = cpool.tile([128, 1], FP, name="eps_sb")
            nc.vector.memset(eps_sb[:], EPS)
            ones_sb = cpool.tile([1, 128], FPR, name="ones_sb")
            nc.sync.dma_start(out=ones_sb[:], in_=onesr[:, :])
            _carrier(nc, ident_sb[:, 0:64])

            emb_sb = epool.tile([128, GROUPS, D], FPR, name="emb_sb")
            for g in range(GROUPS):
                nc.sync.dma_start(out=emb_sb[:, g, :], in_=embr[:, g, :])

            bd_all = bdpool.tile([128, GROUPS, 128], FPR, name="bd_all")
            nc.vector.memset(bd_all[:].bitcast(FP), 0.0)
            gs_all = gspool.tile([128, GROUPS, 128], FPR, name="gs_all")

            def emit_transposes(g):
                e = emb_sb[:, g, :]
                _carrier(nc, e[:, 0:64])
                ets = etpool.tile([128, D], BF, name="ets", tag="ets")
                QW = D // 4
                for qi in range(4):
                    etq = tqpool.tile([128, QW], FPR, name="etq", tag="tq")
                    for ci in range(QW // 128):
                        c = qi * (QW // 128) + ci
                        nc.tensor.transpose(
                            out=etq[:, ci * 128:(ci + 1) * 128],
                            in_=e[:, c * 128:(c + 1) * 128],
                            identity=ident_sb[:],
                        )
                    dst = ets[:, qi * QW:(qi + 1) * QW]
                    if qi % 2 == 0:
                        nc.vector.tensor_copy(dst, etq[:])
                    else:
                        nc.scalar.copy(dst, etq[:])
                return ets

            def emit_grams(g, ets):
                _carrier(nc, ets[:, 0:128].bitcast(FP))
                _carrier(nc, ets[:, D // 2:D // 2 + 128].bitcast(FP))
                gp = gppool.tile([128, 128], FP, name="gp", tag="gp")
                for c in range(NCHUNK):
                    nc.tensor.matmul(
                        gp[:],
                        ets[:, c * 128:(c + 1) * 128],
                        ets[:, c * 128:(c + 1) * 128],
                        start=(c == 0),
                        stop=(c == NCHUNK - 1),
                    )
                nc.vector.tensor_copy(gs_all[:, g, :], gp[:])

            def emit_gram_phase(t0):
                pend = (t0, emit_transposes(t0))
                for g in range(t0 + 1, t0 + GPT):
                    ets = emit_transposes(g)
                    emit_grams(*pend)
                    pend = (g, ets)
                emit_grams(*pend)

            def small_a_dma(t):
                """gather diag blocks -> gvec (DMA only)."""
                t0 = t * GPT
                # BIR allows partition steps only in an SBUF AP's outermost
                # dim, and the diagonal couples partition+byte on the SBUF
                # side in one direction, so bounce the 32 KB of diagonal
                # blocks through DRAM: 16 strided writes (one per sample-in-
                # group, partition step legally outermost) + one row load.
                for i in range(GRP):
                    srcg = bass.AP(
                        gs_all.tensor,
                        gs_all.offset + i * (8 * GROUPS * 128)
                        + t0 * 128 + 8 * i,
                        [[GROUPS * 128, 8], [128, GPT], [1, 8]],
                    )
                    dstg = bass.AP(
                        gvd, (t * 128 + i) * 64,
                        [[8, 8], [16 * 64, GPT], [1, 8]],
                    )
                    eng = (nc.sync, nc.sync, nc.gpsimd)[i % 3] \
                        if QUEUE_SPLIT else nc.sync
                    eng.dma_start(out=dstg, in_=srcg)
                gvec = smpool.tile([128, 64], FPR, name="gvec", tag="gvec")
                nc.sync.dma_start(out=gvec[:],
                                  in_=gvd[t * 128:(t + 1) * 128, :])
                return gvec

            def small_a_pe(t, gvec):
                """gvec -> gvt -> scores (PSUM)."""
                gvtT = psmall.tile([64, 128], FPR, name="gvtT", tag="psm")
                _carrier(nc, gvec[:, 0:64])
                nc.tensor.transpose(
                    out=gvtT[:],
                    in_=gvec[:],
                    identity=ident_sb[:],
                )
                gvt = smpool.tile([128, 128], FPR, name="gvt", tag="gvt")
                nc.vector.tensor_copy(gvt[0:64, :], gvtT[:])
                nc.vector.tensor_copy(gvt[64:65, :], ones_sb[0:1, :])
                scores = psmall.tile([128, 256], FP, name="scores", tag="psm")
                _carrier(nc, gvt[:, 0:64])
                nc.tensor.matmul(
                    scores[:],
                    gvt[0:65, :],
                    w2_sb[0:65, :],
                    start=True, stop=True,
                )
                return scores

            def small_b(t, scores):
                """instancenorm stats + fused exp + softmax divide (no PE)."""
                mu = smpool.tile([128, 4], FP, name="mu", tag="mu")
                nc.vector.tensor_reduce(
                    out=mu[:],
                    in_=scores[:].rearrange("p (h x) -> p h x", h=4),
                    axis=AX.X, op=OP.add,
                )
                sq = smpool.tile([128, 4], FP, name="sq", tag="sq")
                pn = smpool.tile([128, 256], FPR, name="pn", tag="pn")
                probs = smpool.tile([128, 256], FP, name="probs", tag="probs")
                for h in range(4):
                    # probs scratch <- x^2 ; sq[:, h] <- sum x^2 per head
                    nc.scalar.activation(
                        out=probs[:, h * 64:(h + 1) * 64],
                        in_=scores[:, h * 64:(h + 1) * 64],
                        func=AF.Square,
                        accum_out=sq[:, h:h + 1],
                    )
                m2 = smpool.tile([128, 4], FP, name="m2", tag="m2")
                # m2 = (mu/4096) * mu = mean^2 ; var = sq/64 - m2
                nc.vector.scalar_tensor_tensor(
                    out=m2[:], in0=mu[:], scalar=1.0 / 4096.0, in1=mu[:],
                    op0=OP.mult, op1=OP.mult,
                )
                var = smpool.tile([128, 4], FP, name="var", tag="var")
                nc.vector.scalar_tensor_tensor(
                    out=var[:], in0=sq[:], scalar=1.0 / 64.0, in1=m2[:],
                    op0=OP.mult, op1=OP.subtract,
                )
                std = smpool.tile([128, 4], FP, name="std", tag="std")
                nc.scalar.activation(out=std[:], in_=var[:], func=AF.Sqrt,
                                     bias=eps_sb[:, 0:1])
                r = smpool.tile([128, 4], FP, name="r", tag="r")
                nc.vector.reciprocal(r[:], std[:])
                rb = smpool.tile([128, 4], FP, name="rb", tag="rb")
                # rb = (mu * -1/64) * r = -mu_mean * r
                nc.vector.scalar_tensor_tensor(
                    out=rb[:], in0=mu[:], scalar=-1.0 / 64.0, in1=r[:],
                    op0=OP.mult, op1=OP.mult,
                )
                for h in range(4):
                    nc.scalar.activation(
                        out=probs[:, h * 64:(h + 1) * 64],
                        in_=scores[:, h * 64:(h + 1) * 64],
                        func=AF.Exp,
                        bias=rb[:, h:h + 1], scale=r[:, h:h + 1],
                    )
                z = smpool.tile([128, 32], FP, name="z", tag="z")
                nc.vector.tensor_reduce(
                    out=z[:],
                    in_=probs[:].rearrange("p (w s) -> p w s", w=32),
                    axis=AX.X, op=OP.add,
                )
                rz = smpool.tile([128, 32], FP, name="rz", tag="rz")
                nc.vector.reciprocal(rz[:], z[:])
                nc.vector.tensor_tensor(
                    out=pn[:].rearrange("p (w s) -> p w s", w=32),
                    in0=probs[:].rearrange("p (w s) -> p w s", w=32),
                    in1=rz[:].unsqueeze(-1).broadcast_to((128, 32, 8)),
                    op=OP.mult,
                )
                return pn

            def small_c(t, pn):
                """probs^T -> M-vectors -> diag scatter -> BD loads."""
                t0 = t * GPT
                pt = smpool.tile([128, 256], FPR, name="pt", tag="pt")
                for half in range(2):
                    ptT = psmall.tile([128, 128], FPR, name="ptT", tag="psm")
                    _carrier(nc, pn[:, half * 128:half * 128 + 64])
                    nc.tensor.transpose(
                        out=ptT[:],
                        in_=pn[:, half * 128:(half + 1) * 128],
                        identity=ident_sb[:],
                    )
                    nc.vector.tensor_copy(
                        pt[:, half * 128:(half + 1) * 128], ptT[:])
                mvp = psmall.tile([128, 64], FP, name="mvp", tag="psm")
                _carrier(nc, pt[:, 0:64])
                _carrier(nc, pt[:, 128:192])
                for half in range(2):
                    nc.tensor.matmul(
                        mvp[:],
                        pt[:, half * 128:(half + 1) * 128],
                        w3_sb[:, half * 64:(half + 1) * 64],
                        start=(half == 0), stop=(half == 1),
                    )
                mv = smpool.tile([128, 64], FPR, name="mv", tag="mv")
                nc.vector.tensor_copy(mv[:], mvp[:])
                # mv bounces through DRAM; 16 strided loads then drop the
                # diagonal 8x8 blocks into the once-zeroed bd_all (partition
                # step legally outermost on the SBUF destination side).
                nc.sync.dma_start(out=mvd[t * 128:(t + 1) * 128, :], in_=mv[:])
                for b in range(GRP):
                    srcs = bass.AP(
                        mvd, (t * 128 + b) * 64,
                        [[8, 8], [1024, GPT], [1, 8]],
                    )
                    dsts = bass.AP(
                        bd_all.tensor,
                        bd_all.offset + b * (8 * GROUPS * 128)
                        + t0 * 128 + 8 * b,
                        [[GROUPS * 128, 8], [128, GPT], [1, 8]],
                    )
                    eng = (nc.sync, nc.gpsimd, nc.scalar)[b % 3] \
                        if QUEUE_SPLIT else nc.sync
                    eng.dma_start(out=dsts, in_=srcs)

            def emit_apply(g0, g1):
                for g in range(g0, g1):
                    os = opool.tile([128, D], F16, name="os", tag="os")
                    _carrier(nc, bd_all[:, g, 0:64])
                    for jj in range(4):
                        ap_ps = tqpool.tile([128, 512], FP, name="ap_ps",
                                            tag="tq")
                        nc.tensor.matmul(
                            ap_ps[:],
                            bd_all[:, g, :],
                            emb_sb[:, g, jj * 512:(jj + 1) * 512],
                            start=True, stop=True,
                        )
                        dst = os[:, jj * 512:(jj + 1) * 512]
                        if jj % 2 == 0:
                            nc.vector.tensor_copy(dst, ap_ps[:])
                        else:
                            nc.scalar.copy(dst, ap_ps[:])
                    oeng = nc.gpsimd if POOL_OUTS else nc.sync
                    oeng.dma_start(out=outr[:, g, :], in_=os[:])

            # Interleave: the per-tile small-math chain is mostly DMA/DVE/
            # ACT latency, and the gram phase is input-bandwidth gated with
            # PE slack; thread the chain stages AND tile-0 apply groups
            # between the tile-1 gram groups so every engine's in-order
            # queue always has ready work.
            mark("gramT0")
            emit_gram_phase(0)
            pend = (GPT, emit_transposes(GPT))
            gvec0 = scores0 = pn0 = None
            for g in range(GPT + 1, 2 * GPT):
                ets = emit_transposes(g)
                emit_grams(*pend)
                pend = (g, ets)
                if g == GPT + 1:
                    mark("a_dma0")
                    gvec0 = small_a_dma(0)
                elif g == GPT + 4:
                    mark("a_pe0")
                    scores0 = small_a_pe(0, gvec0)
                elif g == GPT + 5:
                    mark("b0")
                    pn0 = small_b(0, scores0)
                elif g == GPT + 6:
                    mark("c0")
                    small_c(0, pn0)
                elif g == GPT + 7:
                    mark("ap02")
                    emit_apply(0, 2)
            emit_grams(*pend)
            mark("a_dma1")
            gvec1 = small_a_dma(1)
            mark("ap24")
            emit_apply(2, 6)
            mark("a_pe1")
            scores1 = small_a_pe(1, gvec1)
            mark("ap68")
            emit_apply(6, GPT)
            mark("b1")
            pn1 = small_b(1, scores1)
            mark("c1")
            small_c(1, pn1)
            mark("apT1")
            emit_apply(GPT, 2 * GPT)
    return _strip_self_waits(nc)


def _host_consts(Wq, Wk, Wv, Wo, rel_table):
    scale = np.float32(1.0 / np.sqrt(T))
    w2 = scale * np.einsum("hta,hsb->abhts", Wq, Wk)
    w2aug = np.zeros((65, 256), np.float32)
    w2aug[:64] = w2.reshape(64, 256)
    idx = np.arange(T)[:, None] - np.arange(T)[None, :] + T - 1
    bias = rel_table[idx]                       # [t, s, H]
    w2aug[64] = bias.transpose(2, 0, 1).reshape(256)
    # w3[64h+8u+q, 8s+t] = Wo[t,u]*Wv[h,q,s]/H
    w3 = np.einsum("tu,hqs->huqst", Wo, Wv).reshape(256, 64) / np.float32(H)
    w3c = np.zeros((128, 128), np.float32)
    w3c[:, 0:64] = w3[0:128]
    w3c[:, 64:128] = w3[128:256]
    return w2aug.astype(np.float32), w3c.astype(np.float32)


def kernel(emb, Wq, Wk, Wv, Wo, rel_table):
    emb = np.ascontiguousarray(emb, dtype=np.float32)
    Wq = np.asarray(Wq, np.float32)
    Wk = np.asarray(Wk, np.float32)
    Wv = np.asarray(Wv, np.float32)
    Wo = np.asarray(Wo, np.float32)
    rel_table = np.asarray(rel_table, np.float32)

    embc = emb.reshape(NCORES, ROWS, D)
    ident = np.eye(128, dtype=np.float32)
    onesr_np = np.ones((1, 128), np.float32)
    w2aug, w3c = _host_consts(Wq, Wk, Wv, Wo, rel_table)
    core_ids = list(range(NCORES))

    del LAST_EXEC_NS[:]
    ncf = _build_fused()
    r = run_bass_kernel_spmd(
        ncf,
        [{"emb": embc[i], "ident": ident, "w2aug": w2aug, "w3c": w3c,
          "onesr": onesr_np}
         for i in range(NCORES)],
        core_ids,
        trace=PROFILE,
    )
    if PROFILE:
        LAST_EXEC_NS.append(r.exec_time_ns)
    out = np.stack([np.asarray(r.results[i]["outp"]) for i in range(NCORES)])
    return out.reshape(N, T, D).astype(np.float32)


# revision 50
# speedup vs baseline: 1.8679x; 1.0781x over previous
"""Trainium2 Bass kernel for nn_Attention_org_10514079941402 (fused 1-launch).

Math per sample n (emb[n] is [T=8, D=2048]):
  G[n]      = emb[n] @ emb[n].T                       (8x8 Gram, contracts D)
  scores[h] = Wq[h] @ G[n] @ Wk[h].T / sqrt(T) + bias[h]
  probs     = softmax(instancenorm(scores))           (mean-shift drops out of
                                                       softmax; only the 1/std
                                                       scale + mu*r bias matter)
  M[n]      = (1/H) * Wo @ (sum_h probs[h] @ Wv[h])   (8x8)
  out[n]    = M[n] @ emb[n]

Everything runs in ONE device launch per core (data parallel over N):
  1. emb resident in SBUF; per 128-row group: PE transposes (fp32r) ->
     bf16 Et -> PE gram -> gs_all.
  2. Per 128-sample tile: diagonal 8x8 blocks of the group-grams are pulled
     into a per-sample layout gvec[sample, 64] with 16 pure-stride SBUF->SBUF
     DMAs (one per sample-in-group index; BIR forbids APs whose dims couple
     partition+byte strides, but fixing i per DMA keeps every stride pure).
     scores = W2aug^T @ [gvec|1] on PE (the bias rides as a 65th contraction
     row); instancenorm stats + exp fused into 4 ACT activations
     (exp(x*r - mu*r)); softmax row-sums on DVE; probs^T via PE; M-vectors
     via 2 accumulating PE matmuls against W3; scattered into a zeroed
     block-diagonal BD_all with 16 more pure-stride DMAs.
  3. Per group: out = BD^T @ emb on PE in fp32r, written out as fp16
     (halves the store traffic; host casts back to fp32).

Walrus constraint: a PE instruction carries at most ONE sync wait. bf16
LDWEIGHTS instructions are inserted as pure wait-carriers; any instruction
still carrying >=2 waits gets the extras hoisted onto Drain instructions.
"""

import numpy as np

import concourse.bass as bass
import concourse.mybir as mybir
import concourse.tile as tile
from concourse.bass_utils import run_bass_kernel_spmd

PROFILE = False          # set by test harness
LAST_EXEC_NS = []        # per-launch HW exec times when PROFILE
QUEUE_SPLIT = True       # spread tiny-DMA issue across SP/Pool/ACT queues
POOL_OUTS = False        # issue output DMAs from the Pool (SWDGE) queue

N, T, D, H = 2048, 8, 2048, 4
NCORES = 8
NPC = N // NCORES            # 256 samples per core
GRP = 16                     # samples per 128-row group
GROUPS = NPC // GRP          # 16 groups per core
GPT = 8                      # groups per 128-sample tile
TILES = GROUPS // GPT        # 2 tiles per core
ROWS = NPC * T               # 2048 rows per core
EPS = 1e-5
FP = mybir.dt.float32
FPR = mybir.dt.float32r
BF = mybir.dt.bfloat16
F16 = mybir.dt.float16
NCHUNK = D // 128            # 16 transpose/gram chunks per group


def _carrier(nc, ap64):
    """bf16 LDWEIGHTS reading ap64 (a [128, 64] fp32 slice): absorbs the
    producer's semaphore wait onto a write-free PE instruction."""
    nc.tensor.ldweights(ap64.bitcast(BF))


def _strip_self_waits(nc):
    """Walrus accepts only ONE sync wait per engine instruction.

    1. Tile emits same-engine self-waits for slot releases; on strict-FIFO
       engines (DVE, ACT) program order already guarantees them - drop them.
    2. Any instruction still carrying >=2 waits gets the extras hoisted onto
       single-wait Drain instructions inserted just before it (same engine).
    """
    pref = {"EngineType.DVE": "DVE", "EngineType.ACT": "ACT",
            "EngineType.Activation": "ACT"}
    for blk in nc.m.functions[0].blocks:
        idx = 0
        insts = blk.instructions
        while idx < len(insts):
            inst = insts[idx]
            si = inst.sync_info
            if si is None:
                idx += 1
                continue
            waits = list(si.on_wait)
            if len(waits) < 2:
                idx += 1
                continue
            p = pref.get(str(inst.engine))
            if p is not None:
                keep = [w for w in waits if not w.ant_name.startswith(p)]
                if 1 <= len(keep) < len(waits):
                    waits = keep
            if len(waits) >= 2:
                for k, w in enumerate(waits[:-1]):
                    d = mybir.InstDrain(
                        name=f"{inst.name}_w{k}", ins=[], outs=[],
                        sync_info=mybir.SyncInfo(on_wait=[w], on_update=[]),
                    )
                    d.engine = inst.engine
                    insts.insert(idx, d)
                    idx += 1
                waits = [waits[-1]]
            inst.sync_info = mybir.SyncInfo(
                on_wait=waits, on_update=list(si.on_update)
            )
            idx += 1
    return nc


def _build_fused(marks=None):
    def mark(label):
        if marks is not None:
            marks.append((label, nc.next_id()))
    nc = bass.Bass()
    emb = nc.dram_tensor("emb", [ROWS, D], FPR, kind="ExternalInput")
    ident = nc.dram_tensor("ident", [128, 128], FPR, kind="ExternalInput")
    w2aug = nc.dram_tensor("w2aug", [65, 256], FPR, kind="ExternalInput")
    w3c = nc.dram_tensor("w3c", [128, 128], FPR, kind="ExternalInput")
    outp = nc.dram_tensor("outp", [ROWS, D], F16, kind="ExternalOutput")
    gvd = nc.dram_tensor("gvd", [NPC, 64], FPR, kind="Internal")
    mvd = nc.dram_tensor("mvd", [NPC, 64], FPR, kind="Internal")
    onesr = nc.dram_tensor("onesr", [1, 128], FPR, kind="ExternalInput")
    embr = emb[:, :].rearrange("(g p) d -> p g d", p=128)   # [128, GROUPS, D]
    outr = outp[:, :].rearrange("(g p) d -> p g d", p=128)
    AX = mybir.AxisListType
    OP = mybir.AluOpType
    AF = mybir.ActivationFunctionType

    with tile.TileContext(nc) as tc:
        with tc.tile_pool(name="const", bufs=1) as cpool, \
             tc.tile_pool(name="embp", bufs=1) as epool, \
             tc.tile_pool(name="etp", bufs=2) as etpool, \
             tc.tile_pool(name="gsp", bufs=1) as gspool, \
             tc.tile_pool(name="bdp", bufs=1) as bdpool, \
             tc.tile_pool(name="sml", bufs=2) as smpool, \
             tc.tile_pool(name="osb", bufs=6) as opool, \
             tc.tile_pool(name="tq", bufs=5, space="PSUM") as tqpool, \
             tc.tile_pool(name="gp", bufs=1, space="PSUM") as gppool, \
             tc.tile_pool(name="psm", bufs=2, space="PSUM") as psmall:
            ident_sb = cpool.tile([128, 128], FPR, name="ident_sb")
            nc.sync.dma_start(out=ident_sb[:], in_=ident[:, :])
            w2_sb = cpool.tile([128, 256], FPR, name="w2_sb")
            nc.sync.dma_start(out=w2_sb[0:65, :], in_=w2aug[:, :])
            w3_sb = cpool.tile([128, 128], FPR, name="w3_sb")
            nc.sync.dma_start(out=w3_sb[:], in_=w3c[:, :])
            eps_sb = cpool.tile([128, 1], FP, name="eps_sb")
            nc.vector.memset(eps_sb[:], EPS)
            ones_sb = cpool.tile([1, 128], FPR, name="ones_sb")
            nc.sync.dma_start(out=ones_sb[:], in_=onesr[:, :])
            _carrier(nc, ident_sb[:, 0:64])

            emb_sb = epool.tile([128, GROUPS, D], FPR, name="emb_sb")
            for g in range(GROUPS):
                nc.sync.dma_start(out=emb_sb[:, g, :], in_=embr[:, g, :])

            bd_all = bdpool.tile([128, GROUPS, 128], FPR, name="bd_all")
            nc.vector.memset(bd_all[:].bitcast(FP), 0.0)
            gs_all = gspool.tile([128, GROUPS, 128], FPR, name="gs_all")

            def emit_transposes(g):
                e = emb_sb[:, g, :]
                _carrier(nc, e[:, 0:64])
                ets = etpool.tile([128, D], BF, name="ets", tag="ets")
                QW = D // 4
                for qi in range(4):
                    etq = tqpool.tile([128, QW], FPR, name="etq", tag="tq")
                    for ci in range(QW // 128):
                        c = qi * (QW // 128) + ci
                        nc.tensor.transpose(
                            out=etq[:, ci * 128:(ci + 1) * 128],
                            in_=e[:, c * 128:(c + 1) * 128],
                            identity=ident_sb[:],
                        )
                    dst = ets[:, qi * QW:(qi + 1) * QW]
                    if qi % 2 == 0:
                        nc.vector.tensor_copy(dst, etq[:])
                    else:
                        nc.scalar.copy(dst, etq[:])
                return ets

            def emit_grams(g, ets):
                _carrier(nc, ets[:, 0:128].bitcast(FP))
                _carrier(nc, ets[:, D // 2:D // 2 + 128].bitcast(FP))
                gp = gppool.tile([128, 128], FP, name="gp", tag="gp")
                for c in range(NCHUNK):
                    nc.tensor.matmul(
                        gp[:],
                        ets[:, c * 128:(c + 1) * 128],
                        ets[:, c * 128:(c + 1) * 128],
                        start=(c == 0),
                        stop=(c == NCHUNK - 1),
                    )
                nc.vector.tensor_copy(gs_all[:, g, :], gp[:])

            def emit_gram_phase(t0):
                pend = (t0, emit_transposes(t0))
                for g in range(t0 + 1, t0 + GPT):
                    ets = emit_transposes(g)
                    emit_grams(*pend)
                    pend = (g, ets)
                emit_grams(*pend)

            def small_a_dma(t):
                """gather diag blocks -> gvec (DMA only)."""
                t0 = t * GPT
                # BIR allows partition steps only in an SBUF AP's outermost
                # dim, and the diagonal couples partition+byte on the SBUF
                # side in one direction, so bounce the 32 KB of diagonal
                # blocks through DRAM: 16 strided writes (one per sample-in-
                # group, partition step legally outermost) + one row load.
                for i in range(GRP):
                    srcg = bass.AP(
                        gs_all.tensor,
                        gs_all.offset + i * (8 * GROUPS * 128)
                        + t0 * 128 + 8 * i,
                        [[GROUPS * 128, 8], [128, GPT], [1, 8]],
                    )
                    dstg = bass.AP(
                        gvd, (t * 128 + i) * 64,
                        [[8, 8], [16 * 64, GPT], [1, 8]],
                    )
                    eng = (nc.sync, nc.sync, nc.gpsimd)[i % 3] \
                        if QUEUE_SPLIT else nc.sync
                    eng.dma_start(out=dstg, in_=srcg)
                gvec = smpool.tile([128, 64], FPR, name="gvec", tag="gvec")
                nc.sync.dma_start(out=gvec[:],
                                  in_=gvd[t * 128:(t + 1) * 128, :])
                return gvec

            def small_a_pe(t, gvec):
                """gvec -> gvt -> scores (PSUM)."""
                gvtT = psmall.tile([64, 128], FPR, name="gvtT", tag="psm")
                _carrier(nc, gvec[:, 0:64])
                nc.tensor.transpose(
                    out=gvtT[:],
                    in_=gvec[:],
                    identity=ident_sb[:],
                )
                gvt = smpool.tile([128, 128], FPR, name="gvt", tag="gvt")
                nc.vector.tensor_copy(gvt[0:64, :], gvtT[:])
                nc.vector.tensor_copy(gvt[64:65, :], ones_sb[0:1, :])
                scores = psmall.tile([128, 256], FP, name="scores", tag="psm")
                _carrier(nc, gvt[:, 0:64])
                nc.tensor.matmul(
                    scores[:],
                    gvt[0:65, :],
                    w2_sb[0:65, :],
                    start=True, stop=True,
                )
                return scores

            def small_b(t, scores):
                """instancenorm stats + fused exp + softmax divide (no PE)."""
                mu = smpool.tile([128, 4], FP, name="mu", tag="mu")
                nc.vector.tensor_reduce(
                    out=mu[:],
                    in_=scores[:].rearrange("p (h x) -> p h x", h=4),
                    axis=AX.X, op=OP.add,
                )
                sq = smpool.tile([128, 4], FP, name="sq", tag="sq")
                pn = smpool.tile([128, 256], FPR, name="pn", tag="pn")
                probs = smpool.tile([128, 256], FP, name="probs", tag="probs")
                for h in range(4):
                    # probs scratch <- x^2 ; sq[:, h] <- sum x^2 per head
                    nc.scalar.activation(
                        out=probs[:, h * 64:(h + 1) * 64],
                        in_=scores[:, h * 64:(h + 1) * 64],
                        func=AF.Square,
                        accum_out=sq[:, h:h + 1],
                    )
                m2 = smpool.tile([128, 4], FP, name="m2", tag="m2")
                # m2 = (mu/4096) * mu = mean^2 ; var = sq/64 - m2
                nc.vector.scalar_tensor_tensor(
                    out=m2[:], in0=mu[:], scalar=1.0 / 4096.0, in1=mu[:],
                    op0=OP.mult, op1=OP.mult,
                )
                var = smpool.tile([128, 4], FP, name="var", tag="var")
                nc.vector.scalar_tensor_tensor(
                    out=var[:], in0=sq[:], scalar=1.0 / 64.0, in1=m2[:],
                    op0=OP.mult, op1=OP.subtract,
                )
                std = smpool.tile([128, 4], FP, name="std", tag="std")
                nc.scalar.activation(out=std[:], in_=var[:], func=AF.Sqrt,
                                     bias=eps_sb[:, 0:1])
                r = smpool.tile([128, 4], FP, name="r", tag="r")
                nc.vector.reciprocal(r[:], std[:])
                rb = smpool.tile([128, 4], FP, name="rb", tag="rb")
                # rb = (mu * -1/64) * r = -mu_mean * r
                nc.vector.scalar_tensor_tensor(
                    out=rb[:], in0=mu[:], scalar=-1.0 / 64.0, in1=r[:],
                    op0=OP.mult, op1=OP.mult,
                )
                for h in range(4):
                    nc.scalar.activation(
                        out=probs[:, h * 64:(h + 1) * 64],
                        in_=scores[:, h * 64:(h + 1) * 64],
                        func=AF.Exp,
                        bias=rb[:, h:h + 1], scale=r[:, h:h + 1],
                    )
                z = smpool.tile([128, 32], FP, name="z", tag="z")
                nc.vector.tensor_reduce(
                    out=z[:],
                    in_=probs[:].rearrange("p (w s) -> p w s", w=32),
                    axis=AX.X, op=OP.add,
                )
                rz = smpool.tile([128, 32], FP, name="rz", tag="rz")
                nc.vector.reciprocal(rz[:], z[:])
                nc.vector.tensor_tensor(
                    out=pn[:].rearrange("p (w s) -> p w s", w=32),
                    in0=probs[:].rearrange("p (w s) -> p w s", w=32),
                    in1=rz[:].unsqueeze(-1).broadcast_to((128, 32, 8)),
                    op=OP.mult,
                )
                return pn

            def small_c(t, pn):
                """probs^T -> M-vectors -> diag scatter -> BD loads."""
                t0 = t * GPT
                pt = smpool.tile([128, 256], FPR, name="pt", tag="pt")
                for half in range(2):
                    ptT = psmall.tile([128, 128], FPR, name="ptT", tag="psm")
                    _carrier(nc, pn[:, half * 128:half * 128 + 64])
                    nc.tensor.transpose(
                        out=ptT[:],
                        in_=pn[:, half * 128:(half + 1) * 128],
                        identity=ident_sb[:],
                    )
                    nc.vector.tensor_copy(
                        pt[:, half * 128:(half + 1) * 128], ptT[:])
                mvp = psmall.tile([128, 64], FP, name="mvp", tag="psm")
                _carrier(nc, pt[:, 0:64])
                _carrier(nc, pt[:, 128:192])
                for half in range(2):
                    nc.tensor.matmul(
                        mvp[:],
                        pt[:, half * 128:(half + 1) * 128],
                        w3_sb[:, half * 64:(half + 1) * 64],
                        start=(half == 0), stop=(half == 1),
                    )
                mv = smpool.tile([128, 64], FPR, name="mv", tag="mv")
                nc.vector.tensor_copy(mv[:], mvp[:])
                # mv bounces through DRAM; 16 strided loads then drop the
                # diagonal 8x8 blocks into the once-zeroed bd_all (partition
                # step legally outermost on the SBUF destination side).
                nc.sync.dma_start(out=mvd[t * 128:(t + 1) * 128, :], in_=mv[:])
                for b in range(GRP):
                    srcs = bass.AP(
                        mvd, (t * 128 + b) * 64,
                        [[8, 8], [1024, GPT], [1, 8]],
                    )
                    dsts = bass.AP(
                        bd_all.tensor,
                        bd_all.offset + b * (8 * GROUPS * 128)
                        + t0 * 128 + 8 * b,
                        [[GROUPS * 128, 8], [128, GPT], [1, 8]],
                    )
                    eng = (nc.sync, nc.sync, nc.gpsimd)[b % 3] \
                        if QUEUE_SPLIT else nc.sync
                    eng.dma_start(out=dsts, in_=srcs)

            def emit_apply(g0, g1):
                for g in range(g0, g1):
                    os = opool.tile([128, D], F16, name="os", tag="os")
                    _carrier(nc, bd_all[:, g, 0:64])
                    for jj in range(4):
                        ap_ps = tqpool.tile([128, 512], FP, name="ap_ps",
                                            tag="tq")
                        nc.tensor.matmul(
                            ap_ps[:],
                            bd_all[:, g, :],
                            emb_sb[:, g, jj * 512:(jj + 1) * 512],
                            start=True, stop=True,
                        )
                        dst = os[:, jj * 512:(jj + 1) * 512]
                        if jj % 2 == 0:
                            nc.vector.tensor_copy(dst, ap_ps[:])
                        else:
                            nc.scalar.copy(dst, ap_ps[:])
                    oeng = (nc.sync, nc.gpsimd, nc.scalar)[g % 3]
                    oeng.dma_start(out=outr[:, g, :], in_=os[:])

            # Interleave: the per-tile small-math chain is mostly DMA/DVE/
            # ACT latency, and the gram phase is input-bandwidth gated with
            # PE slack; thread the chain stages AND tile-0 apply groups
            # between the tile-1 gram groups so every engine's in-order
            # queue always has ready work.
            mark("gramT0")
            emit_gram_phase(0)
            pend = (GPT, emit_transposes(GPT))
            gvec0 = scores0 = pn0 = None
            for g in range(GPT + 1, 2 * GPT):
                ets = emit_transposes(g)
                emit_grams(*pend)
                pend = (g, ets)
                if g == GPT + 1:
                    mark("a_dma0")
                    gvec0 = small_a_dma(0)
                elif g == GPT + 4:
                    mark("a_pe0")
                    scores0 = small_a_pe(0, gvec0)
                elif g == GPT + 5:
                    mark("b0")
                    pn0 = small_b(0, scores0)
                elif g == GPT + 6:
                    mark("c0")
                    small_c(0, pn0)
                elif g == GPT + 7:
                    mark("ap02")
                    emit_apply(0, 2)
            emit_grams(*pend)
            mark("a_dma1")
            gvec1 = small_a_dma(1)
            mark("a_pe1")
            scores1 = small_a_pe(1, gvec1)
            mark("b1")
            pn1 = small_b(1, scores1)
            mark("c1")
            small_c(1, pn1)
            mark("ap68")
            emit_apply(2, GPT)
            mark("apT1")
            emit_apply(GPT, 2 * GPT)
    return _strip_self_waits(nc)


def _host_consts(Wq, Wk, Wv, Wo, rel_table):
    scale = np.float32(1.0 / np.sqrt(T))
    w2 = scale * np.einsum("hta,hsb->abhts", Wq, Wk)
    w2aug = np.zeros((65, 256), np.float32)
    w2aug[:64] = w2.reshape(64, 256)
    idx = np.arange(T)[:, None] - np.arange(T)[None, :] + T - 1
    bias = rel_table[idx]                       # [t, s, H]
    w2aug[64] = bias.transpose(2, 0, 1).reshape(256)
    # w3[64h+8u+q, 8s+t] = Wo[t,u]*Wv[h,q,s]/H
    w3 = np.einsum("tu,hqs->huqst", Wo, Wv).reshape(256, 64) / np.float32(H)
    w3c = np.zeros((128, 128), np.float32)
    w3c[:, 0:64] = w3[0:128]
    w3c[:, 64:128] = w3[128:256]
    return w2aug.astype(np.float32), w3c.astype(np.float32)


def kernel(emb, Wq, Wk, Wv, Wo, rel_table):
    emb = np.ascontiguousarray(emb, dtype=np.float32)
    Wq = np.asarray(Wq, np.float32)
    Wk = np.asarray(Wk, np.float32)
    Wv = np.asarray(Wv, np.float32)
    Wo = np.asarray(Wo, np.float32)
    rel_table = np.asarray(rel_table, np.float32)

    embc = emb.reshape(NCORES, ROWS, D)
    ident = np.eye(128, dtype=np.float32)
    onesr_np = np.ones((1, 128), np.float32)
    w2aug, w3c = _host_consts(Wq, Wk, Wv, Wo, rel_table)
    core_ids = list(range(NCORES))

    del LAST_EXEC_NS[:]
    ncf = _build_fused()
    r = run_bass_kernel_spmd(
        ncf,
        [{"emb": embc[i], "ident": ident, "w2aug": w2aug, "w3c": w3c,
          "onesr": onesr_np}
         for i in range(NCORES)],
        core_ids,
        trace=PROFILE,
    )
    if PROFILE:
        LAST_EXEC_NS.append(r.exec_time_ns)
    out = np.stack([np.asarray(r.results[i]["outp"]) for i in range(NCORES)])
    return out.reshape(N, T, D).astype(np.float32)
